# revision 15
# baseline (speedup 1.0000x reference)
"""BertSelfAttention on 8 Trainium2 NeuronCores (Bass/Tile).

Sharding: tensor-parallel over heads. 16 heads / 8 cores = 2 heads (128
head-dim columns) per core. Each core computes the Q/K/V projections for
its 128 output dims over all 4096 tokens, then attention for its 2 heads
over both batches, producing a [128, 4096] d-major slice of the output.
The host transposes hidden_states once, feeds every core the same
[1024, 4096] activation matrix plus its private weight slice, and
reassembles the full [2, 2048, 1024] output from the 8 slices.

Device-side layout choices (all driven by "the PE contracts over the
partition dim" + "softmax reductions are only cheap along the free dim is
false here -- we need them along keys"):
  - Q,K,V are produced d-major: [dim, token]. QK^T then yields scores
    TRANSPOSED, S^T[key, query], with keys on partitions.
  - exp() runs on ScalarE with the 1/sqrt(64) scale folded in.
  - V is PE-transposed to token-major tiles with a ones-column appended;
    the PV matmul (contracting over keys) then computes both the context
    AND the softmax denominator (row 64 of the psum) in one pass.
  - The denominator reciprocal is broadcast across partitions with a
    K=1 matmul against a ones row, then one DVE multiply normalizes.
"""

import numpy as np

import concourse.bass as bass
import concourse.tile as tile
from concourse import bacc, mybir
from concourse.bass_utils import run_bass_kernel_spmd
from concourse.masks import make_identity

# Problem shape (hardcoded; harness contract)
B, S, H = 2, 2048, 1024
NUM_HEADS, DH = 16, 64
NCORES = 8
T = B * S                 # 4096 tokens total
D = H // NCORES           # 128 output dims per core (2 heads)
KC = H // 128             # 8 contraction chunks for projections
QB = 512                  # query-block width (one psum bank)
NQB = S // QB             # 4 query blocks per batch
NTB = T // QB             # 8 projection token-blocks
NKT = S // 128            # 16 key tiles per batch
KTC = 2                   # key tiles per exp chunk (psum-budget limited)
SCALE = 1.0 / float(np.sqrt(DH))

F32 = mybir.dt.float32
F32R = mybir.dt.float32r
EXP = mybir.ActivationFunctionType.Exp


def _r(ap):
    """View an fp32 AP as float32r for full-rate PE matmuls."""
    return ap.bitcast(F32R)


def build(use_mask: bool, use_bias: bool, fast_mm: bool = True):
    r = _r if fast_mm else (lambda ap: ap)
    nc = bacc.Bacc("TRN2", target_bir_lowering=False)

    hT = nc.dram_tensor("hT", [H, T], F32, kind="ExternalInput")
    wq = nc.dram_tensor("wq", [H, D], F32, kind="ExternalInput")
    wk = nc.dram_tensor("wk", [H, D], F32, kind="ExternalInput")
    wv = nc.dram_tensor("wv", [H, D], F32, kind="ExternalInput")
    if use_bias:
        bq = nc.dram_tensor("bq", [D, 1], F32, kind="ExternalInput")
        bk = nc.dram_tensor("bk", [D, 1], F32, kind="ExternalInput")
        bv = nc.dram_tensor("bv", [D, 1], F32, kind="ExternalInput")
    if use_mask:
        # host pre-transposes to [128, B, NKT] so the DMA is contiguous
        mask = nc.dram_tensor("mask", [128, B, NKT], F32, kind="ExternalInput")
    out = nc.dram_tensor("out", [D, T], F32, kind="ExternalOutput")

    with tile.TileContext(nc) as tc:
        with (
            tc.tile_pool(name="consts", bufs=1) as consts,
            tc.tile_pool(name="qkv", bufs=1) as qkvp,
            tc.tile_pool(name="ht", bufs=2) as htp,
            tc.tile_pool(name="vtm", bufs=2) as vtmp,
            tc.tile_pool(name="e", bufs=6) as ep,
            tc.tile_pool(name="small", bufs=4) as smallp,
            tc.tile_pool(name="pp", bufs=2, space="PSUM") as pp,
            tc.tile_pool(name="qk", bufs=2, space="PSUM") as qkp,
            tc.tile_pool(name="pv", bufs=2, space="PSUM") as pvp,
        ):
            # ---- constants ----
            w_sb = {}
            b_sb = {}
            for name, wt in (("q", wq), ("k", wk), ("v", wv)):
                w_t = consts.tile([128, KC, D], F32R, tag=f"w{name}")
                nc.sync.dma_start(
                    out=w_t[:],
                    in_=wt.rearrange("(kc p) d -> p kc d", p=128).bitcast(F32R),
                )
                w_sb[name] = w_t
            if use_bias:
                for name, bt in (("q", bq), ("k", bk), ("v", bv)):
                    b_t = consts.tile([128, 1], F32, tag=f"b{name}")
                    nc.sync.dma_start(out=b_t[:], in_=bt[:])
                    b_sb[name] = b_t
            ident = consts.tile([128, 128], F32, tag="ident")
            make_identity(nc, ident[:])
            # ones constants (memset can't encode f32r; stage via f32 + copy)
            ones_st = consts.tile([128, DH], F32, tag="onesst")
            nc.vector.memset(ones_st[:], 1.0)
            # row at partition 64: stationary operand of the reciprocal-
            # broadcast matmuls (K=1 contraction at row 64, dst partitions 0-63)
            ones_row = consts.tile([65, DH], F32R, tag="onesrow")
            nc.vector.tensor_copy(ones_row[64:65, :], ones_st[64:65, :])
            if use_mask:
                mask_sb = consts.tile([128, B, NKT], F32, tag="mask")
                nc.sync.dma_start(out=mask_sb[:], in_=mask[:])

            Qd = qkvp.tile([128, T], F32R, tag="Qd")
            Kd = qkvp.tile([128, T], F32R, tag="Kd")
            Vd = qkvp.tile([128, T], F32, tag="Vd")

            # ---- phase 1: Q/K/V projections, d-major ----
            hT_r = hT.rearrange("(kc p) t -> p kc t", p=128)
            for tb in range(NTB):
                t0 = tb * QB
                ht_t = htp.tile([128, KC, QB], F32R, tag="ht")
                nc.sync.dma_start(
                    out=ht_t[:], in_=hT_r[:, :, t0 : t0 + QB].bitcast(F32R)
                )
                for name, dest in (("q", Qd), ("k", Kd), ("v", Vd)):
                    ps = pp.tile([128, QB], F32, tag="pp")
                    for kc in range(KC):
                        nc.tensor.matmul(
                            ps[:],
                            w_sb[name][:, kc, :],
                            ht_t[:, kc, :],
                            start=(kc == 0),
                            stop=(kc == KC - 1),
                        )
                    if use_bias:
                        nc.vector.tensor_scalar_add(
                            dest[:, t0 : t0 + QB], ps[:], b_sb[name][:]
                        )
                    else:
                        nc.vector.tensor_copy(dest[:, t0 : t0 + QB], ps[:])

            # ---- per batch: V transpose then attention ----
            for b in range(B):
                # V token-major tiles per k-tile, 130 wide:
                # [h0 dims 0-63 | ones | h1 dims 65-128 | ones]; the ones
                # column makes the PV matmul also produce the softmax
                # denominator in psum row 64.
                vtm = vtmp.tile([128, NKT, 130], F32R, tag="vtm")
                nc.vector.tensor_copy(
                    vtm[:, :, 64:65],
                    ones_st[:, 0:NKT].rearrange("p (a o) -> p a o", o=1),
                )
                nc.vector.tensor_copy(
                    vtm[:, :, 129:130],
                    ones_st[:, 0:NKT].rearrange("p (a o) -> p a o", o=1),
                )
                for kt in range(NKT):
                    k0 = b * S + kt * 128
                    tps = pp.tile([128, 128], F32, tag="pp")
                    nc.tensor.transpose(tps[:], Vd[:, k0 : k0 + 128], ident[:])
                    nc.vector.tensor_copy(
                        vtm[:, kt, :].rearrange("p (g c) -> p g c", g=2)[:, :, 0:64],
                        tps.rearrange("p (g c) -> p g c", g=2),
                    )

                for qb in range(NQB):
                    q0 = b * S + qb * QB
                    # per head: one bank [65, QB] = context rows 0-63 +
                    # softmax denominator row 64, one accumulation group
                    ctx_ps = [
                        pvp.tile([65, QB], F32, tag="ctx", name=f"ctx{h}")
                        for h in range(2)
                    ]
                    for ktc in range(NKT // KTC):
                        sps = [
                            qkp.tile([128, KTC, QB], F32, tag="sps", name=f"sps{h}")
                            for h in range(2)
                        ]
                        # QK^T: both heads' matmuls adjacent so their
                        # disjoint PE row-groups (0-63 / 64-127) overlap
                        for j in range(KTC):
                            kt = ktc * KTC + j
                            k0 = b * S + kt * 128
                            for h in (0, 1):
                                nc.tensor.matmul(
                                    sps[h][:, j, :],
                                    Kd[h * 64 : (h + 1) * 64, k0 : k0 + 128],
                                    Qd[h * 64 : (h + 1) * 64, q0 : q0 + QB],
                                    start=True,
                                    stop=True,
                                )
                        ets = []
                        for h in (0, 1):
                            et = ep.tile([128, KTC, QB], F32R, tag="e", name=f"et{h}")
                            if use_mask:
                                for j in range(KTC):
                                    kt = ktc * KTC + j
                                    nc.scalar.activation(
                                        et[:, j, :],
                                        sps[h][:, j, :],
                                        EXP,
                                        bias=mask_sb[:, b, kt : kt + 1],
                                        scale=SCALE,
                                    )
                            else:
                                nc.scalar.activation(
                                    et[:], sps[h][:], EXP, scale=SCALE
                                )
                            ets.append(et)
                        for j in range(KTC):
                            kt = ktc * KTC + j
                            for h in (0, 1):
                                nc.tensor.matmul(
                                    ctx_ps[h][:],
                                    vtm[:, kt, h * 65 : (h + 1) * 65],
                                    ets[h][:, j, :],
                                    start=(kt == 0),
                                    stop=(kt == NKT - 1),
                                )
                    for h in (0, 1):
                        rec = smallp.tile([65, QB], F32R, tag="rec")
                        with nc.allow_low_precision(reason="bcast operand"):
                            nc.vector.reciprocal(
                                rec[64:65, :], ctx_ps[h][64:65, :]
                            )
                        bc = pp.tile([64, QB], F32, tag="pp")
                        nc.tensor.matmul(
                            bc[:],
                            ones_row[64:65, :],
                            rec[64:65, :],
                            start=True,
                            stop=True,
                        )
                        # DVE TensorTensor cannot read two PSUM operands;
                        # stage the broadcast reciprocal through SBUF
                        bc_sb = smallp.tile([64, QB], F32, tag="bcsb")
                        nc.vector.tensor_copy(bc_sb[:], bc[:])
                        ot = smallp.tile([64, QB], F32, tag="ot")
                        nc.vector.tensor_mul(ot[:], ctx_ps[h][0:64, :], bc_sb[:])
                        nc.sync.dma_start(
                            out=out[h * 64 : (h + 1) * 64, q0 : q0 + QB],
                            in_=ot[:],
                        )
    nc.compile()
    return nc


_BUILD_CACHE = {}


def _get_nc(use_mask, use_bias, fast_mm=True):
    key = (use_mask, use_bias, fast_mm)
    if key not in _BUILD_CACHE:
        _BUILD_CACHE[key] = build(use_mask, use_bias, fast_mm)
    return _BUILD_CACHE[key]


def kernel(hidden_states, attention_mask, Wq, bq, Wk, bk, Wv, bv, _trace=False):
    hidden = np.ascontiguousarray(np.asarray(hidden_states, dtype=np.float32))
    mask = np.asarray(attention_mask, dtype=np.float32).reshape(B, S)
    Wq = np.asarray(Wq, dtype=np.float32)
    Wk = np.asarray(Wk, dtype=np.float32)
    Wv = np.asarray(Wv, dtype=np.float32)
    bq = np.asarray(bq, dtype=np.float32)
    bk = np.asarray(bk, dtype=np.float32)
    bv = np.asarray(bv, dtype=np.float32)

    use_mask = bool(np.any(mask != 0.0))
    use_bias = bool(np.any(bq != 0.0) or np.any(bk != 0.0) or np.any(bv != 0.0))
    nc = _get_nc(use_mask, use_bias)

    hT = np.ascontiguousarray(hidden.reshape(T, H).T)  # [H, T]
    in_maps = []
    for c in range(NCORES):
        sl = slice(c * D, (c + 1) * D)
        m = {
            "hT": hT,
            "wq": np.ascontiguousarray(Wq[:, sl]),
            "wk": np.ascontiguousarray(Wk[:, sl]),
            "wv": np.ascontiguousarray(Wv[:, sl]),
        }
        if use_bias:
            m["bq"] = np.ascontiguousarray(bq[sl].reshape(D, 1))
            m["bk"] = np.ascontiguousarray(bk[sl].reshape(D, 1))
            m["bv"] = np.ascontiguousarray(bv[sl].reshape(D, 1))
        if use_mask:
            # [B, S] -> [128, B, NKT]: partition p holds key kt*128+p
            m["mask"] = np.ascontiguousarray(
                mask.reshape(B, NKT, 128).transpose(2, 0, 1)
            )
        in_maps.append(m)

    res = run_bass_kernel_spmd(
        nc, in_maps, core_ids=list(range(NCORES)), trace=_trace
    )
    # assemble: core c's [128, T] d-major slice -> rows c*128:(c+1)*128
    full_dT = np.concatenate([res.results[c]["out"] for c in range(NCORES)], axis=0)
    out = np.ascontiguousarray(full_dT.T).reshape(B, S, H).astype(np.float32)
    if _trace:
        return out, res
    return out
